# revision 48
# baseline (speedup 1.0000x reference)
"""Trainium2 Bass kernel for an attention-GRU cell (Bahdanau attention + GRU update).

Computation (per batch row b):
    x   = inputs @ Wi + bi
    xg  = x @ kernel + bias                       (split into x_z, x_r, x_h)
    q   = h_tm1 @ Ua + ba_u
    S   = tanh(context @ Wa + ba_w + q)           [t, U]
    sc  = S @ Va + ba_v                           [t]
    attn = softmax(sc)                            (scores bounded -> no max-sub)
    cv  = sum_t attn * context                    [U]
    cg  = cv @ attention_kernel                   (c_z, c_r, c_h)
    z   = sigmoid(x_z + h@Rz + c_z) ; r = sigmoid(x_r + h@Rr + c_r)
    hb  = tanh(x_h + (r*h)@Rh + c_h)
    h   = z*h_tm1 + (1-z)*hb ; out = h @ Wo + bo

Sharding: batch (64) split across 8 cores, 8 batches/core, weights replicated.
Each core fully independent (no collectives).  Measured ~150us on HW
(v1 baseline: 315-328us).

Design (what got it from 328us to ~150us):
  - context is uploaded HOST-TRANSPOSED as f8e4 [b, U, T] ("ctxT8").  The
    score matmul and the cv reduction both need ctx^T with u on partitions,
    and both already consumed f8 in the 328us version -- so the entire
    on-chip PE-transpose + PSUM->SBUF unpack pipeline (10us PE + 13us
    ACT/DVE per core) vanishes, and HBM context traffic drops 4x
    (33.6MB f32 -> 8.4MB f8 per core, ~2.9us per batch).
  - scores: S^T per m-chunk in PSUM via f8 DoubleRow matmuls (Wa
    pre-scaled x16 on the host, tanh un-scales via scale=1/16);
    tanh outputs f8 th-tiles with per-partition bias qb = (q + ba_w)^T.
  - sc = S@Va via DoubleRow f8 with a PARTITION-REPLICATED Va stationary
    ([128, m, 128] with identical columns): the score row lands replicated
    across all 128 PSUM partitions at no extra PE cost, so exp reads it
    directly and the normalizer accumulates per partition
    (accum_out) -- no partition_broadcast anywhere.
  - cv partials per t-half on the DVE: scalar_tensor_tensor(natT8 x expRep)
    with accum_out per u-chunk.  (f16 natT for DVE 2x and gpsimd STT were
    both tried and are NOT wins: no 2x on HW for this op, and walrus
    rejects TensorScalarPtr on Pool.)
  - steady state is DVE/ACT-bound at ~13us/batch (DVE: 8 STT = 9.8us;
    ACT: 8 tanh + 2 exp = 11.7us); PE ~9us; DMA ~3us.
  - DMA choreography: ctxT8[b0] in two halves first on the gpsimd SWDGE
    queue, then b1, h/inputs, then the big f16 weights (their descriptors
    queue BEHIND the first context tiles on the shared DMA engines instead
    of racing ahead -- this was worth ~15us of head).  ua/wi ride the
    scalar HWDGE queue (needed first for the qb chain), consts on sync.
  - natT pool bufs=3 and expRep/dump bufs=4: with bufs=2 the natT(b+2)
    dma_start blocks the gpsimd queue on the STT(b) anti-dependency and
    stalls the whole pipeline every other batch (~30us total).
  - gates/posts in two 4-row groups (SBUF compute APs cannot start at
    partition 4): group 0's post is emitted after b4 and overlaps the
    stream; group 1's is the tail (~15us serial GRU chain).  The r-gate
    rides tanh algebra: rh = 0.5*(tanh(rpre/2)+1)*h, with the 0.5 folded
    into the hpre scalar slot.  (AF.Sigmoid exists but forces ACT
    activation-table reloads -- slower.)
  - PSUM: score/psc tiles 3x[128,1024]f32 rotating (6 banks) + 2 banks of
    shared small-matmul tiles.

Numerics: rel err ~4.4e-4 vs the f32 reference (budget 2e-2).  f8e4 on
ctx/Wa/Va/tanh-out is smoothed by the 2048-way softmax average; the gate/
output path stays f16/f32.
"""



import sys

if "/opt/trn_rl_repo" not in sys.path:
    sys.path.insert(0, "/opt/trn_rl_repo")

import numpy as np

import concourse.bass as bass
import concourse.mybir as mybir
import concourse.tile as tile
from concourse import bacc

F32 = mybir.dt.float32
F16 = mybir.dt.float16
F8 = mybir.dt.float8e4
AF = mybir.ActivationFunctionType
OP = mybir.AluOpType
DR = mybir.MatmulPerfMode.DoubleRow

B = 64          # total batch
T = 2048        # context length
U = 512         # units
EMB = 256
NCORES = 8
BPC = B // NCORES   # batches per core
KU = U // 128       # 4 k-chunks over units
TC = T // 128       # 16 t-chunks
TH = 1024           # t positions per half


def _build_program():
    nc = bacc.Bacc("TRN2", target_bir_lowering=False, debug=False, num_devices=NCORES)

    # ---- DRAM I/O ----
    ctx_d = nc.dram_tensor("ctxT8", [BPC, U, T], F8, kind="ExternalInput").ap()
    inp_d = nc.dram_tensor("inp", [BPC, EMB], F32, kind="ExternalInput").ap()
    h0_d = nc.dram_tensor("h0", [BPC, U], F32, kind="ExternalInput").ap()

    wa8_d = nc.dram_tensor("wa8dr", [128, 2, 2, KU, 128], F8, kind="ExternalInput").ap()
    va8_d = nc.dram_tensor("va8rep", [128, KU, 128], F8, kind="ExternalInput").ap()
    id16_d = nc.dram_tensor("ident16", [128, 128], F16, kind="ExternalInput").ap()

    ua_d = nc.dram_tensor("ua16", [U, U], F16, kind="ExternalInput").ap()
    wi_d = nc.dram_tensor("wi16", [EMB, U], F16, kind="ExternalInput").ap()
    kern_d = nc.dram_tensor("kern16", [U, 3 * U], F16, kind="ExternalInput").ap()
    rec_d = nc.dram_tensor("rec16", [U, 3 * U], F16, kind="ExternalInput").ap()
    attk_d = nc.dram_tensor("attk16", [U, 3 * U], F16, kind="ExternalInput").ap()
    wo_d = nc.dram_tensor("wo16", [U, U], F16, kind="ExternalInput").ap()

    bi_d = nc.dram_tensor("bi", [U], F32, kind="ExternalInput").ap()
    bg_d = nc.dram_tensor("biasg", [3 * U], F32, kind="ExternalInput").ap()
    bau_d = nc.dram_tensor("ba_u", [U], F32, kind="ExternalInput").ap()
    bawt_d = nc.dram_tensor("ba_wt8", [128, KU, BPC], F32, kind="ExternalInput").ap()
    bav_d = nc.dram_tensor("ba_v1", [1, 1], F32, kind="ExternalInput").ap()
    bo_d = nc.dram_tensor("bo", [U], F32, kind="ExternalInput").ap()

    out_d = nc.dram_tensor("out_o", [BPC, U], F32, kind="ExternalOutput").ap()
    h_d = nc.dram_tensor("h_o", [BPC, U], F32, kind="ExternalOutput").ap()

    with tile.TileContext(nc) as tc:
        _emit(nc, tc, locals())
    nc.compile()
    return nc


def _bcast_rows(ap_1d, rows, cols):
    """DMA source AP replicating a 1-D [cols] dram tensor across `rows` partitions."""
    return bass.AP(ap_1d.tensor, 0, [[0, rows], [1, cols]])


def _emit(nc, tc, d):
    ctx_d, inp_d, h0_d = d["ctx_d"], d["inp_d"], d["h0_d"]
    wa8_d, va8_d, id16_d = d["wa8_d"], d["va8_d"], d["id16_d"]
    ua_d, wi_d, kern_d, rec_d, attk_d, wo_d = (
        d["ua_d"], d["wi_d"], d["kern_d"], d["rec_d"], d["attk_d"], d["wo_d"],
    )
    bi_d, bg_d, bau_d, bawt_d, bav_d, bo_d = (
        d["bi_d"], d["bg_d"], d["bau_d"], d["bawt_d"], d["bav_d"], d["bo_d"],
    )
    out_d, h_d = d["out_d"], d["h_d"]

    from contextlib import ExitStack

    es = ExitStack()
    wp = es.enter_context(tc.tile_pool(name="weights", bufs=1))
    gp = es.enter_context(tc.tile_pool(name="group", bufs=2))
    bp = es.enter_context(tc.tile_pool(name="perbatch", bufs=3))
    ntp = es.enter_context(tc.tile_pool(name="natT", bufs=3))
    thp = es.enter_context(tc.tile_pool(name="th8", bufs=3))
    erp = es.enter_context(tc.tile_pool(name="exprep", bufs=4))
    # PSUM budget 8 banks: pS 2x[128,1024]f32 = 4 banks; pp (shared small:
    # psT f8 [128,2048] and [8,512]f32 tiles) 2 banks; pR psc_rep 2 banks.
    pS = es.enter_context(tc.tile_pool(name="psS", bufs=3, space="PSUM"))
    pp = es.enter_context(tc.tile_pool(name="psT", bufs=2, space="PSUM"))

    # ---- DMA issue order ----

    def load_kxm(dram, rows, cols, tag, engine):
        t = wp.tile([128, rows // 128, cols], F16, tag=tag, name=tag)
        src = bass.AP(dram.tensor, 0, [[cols, 128], [128 * cols, rows // 128], [1, cols]])
        engine.dma_start(out=t, in_=src)
        return t

    def load_natT(bb, engine=None, split=False):
        t = ntp.tile([128, KU, T], F8, tag="natT", name=f"natT{bb}")
        if split:
            # two half-loads so the first scores can start sooner
            for hh in range(2):
                (engine or nc.gpsimd).dma_start(
                    out=t[:, :, hh * TH:(hh + 1) * TH],
                    in_=bass.AP(ctx_d.tensor, bb * U * T + hh * TH,
                                [[T, 128], [128 * T, KU], [1, TH]]))
        else:
            (engine or nc.gpsimd).dma_start(out=t, in_=bass.AP(
                ctx_d.tensor, bb * U * T, [[T, 128], [128 * T, KU], [1, T]]))
        return t

    # batch 0 first on the gpsimd queue, in two halves so the first score
    # matmuls can start as soon as the th0 half lands.
    nat_pre = {0: load_natT(0, split=True)}

    h016 = wp.tile([BPC, U], F16)
    nc.gpsimd.dma_start(out=h016, in_=h0_d)
    inp16 = wp.tile([BPC, EMB], F16)
    nc.gpsimd.dma_start(out=inp16, in_=inp_d)

    nat_pre[1] = load_natT(1)
    # big weights go on the gpsimd SWDGE queue AFTER the first context
    # tiles: their descriptors then queue behind them on the shared DMA
    # engines instead of racing ahead (saves ~15us of head).
    rec_sb = load_kxm(rec_d, U, 3 * U, "recw", nc.gpsimd)
    kern_sb = load_kxm(kern_d, U, 3 * U, "kernw", nc.gpsimd)
    attk_sb = load_kxm(attk_d, U, 3 * U, "attkw", nc.gpsimd)
    wo_sb = load_kxm(wo_d, U, U, "wow", nc.gpsimd)

    # sync queue: small consts first, big rec/attk last.
    id16 = wp.tile([128, 128], F16)
    nc.sync.dma_start(out=id16, in_=id16_d)
    va8_sb = wp.tile([128, KU, 128], F8)
    nc.sync.dma_start(out=va8_sb, in_=va8_d)
    wa8_sb = wp.tile([128, 2, 2, KU, 128], F8)
    nc.sync.dma_start(out=wa8_sb, in_=wa8_d)
    bavr = wp.tile([128, 1], F32)
    nc.sync.dma_start(out=bavr, in_=bass.AP(bav_d.tensor, 0, [[0, 128], [1, 1]]))
    bawt8 = wp.tile([128, KU, BPC], F32)
    nc.sync.dma_start(out=bawt8, in_=bawt_d)
    bau8 = wp.tile([BPC, U], F32)
    nc.sync.dma_start(out=bau8, in_=_bcast_rows(bau_d, BPC, U))
    bi8 = wp.tile([BPC, U], F32)
    nc.sync.dma_start(out=bi8, in_=_bcast_rows(bi_d, BPC, U))
    bg4 = wp.tile([4, 3 * U], F32)
    nc.sync.dma_start(out=bg4, in_=_bcast_rows(bg_d, 4, 3 * U))
    bo4 = wp.tile([4, U], F32)
    nc.sync.dma_start(out=bo4, in_=_bcast_rows(bo_d, 4, U))
    h032g = []
    for g in range(2):
        t = wp.tile([4, U], F32, tag=f"h032g{g}", name=f"h032g{g}")
        nc.sync.dma_start(out=t, in_=h0_d[g * 4:(g + 1) * 4, :])
        h032g.append(t)

    # scalar(ACT) hwdge queue: ua/wi (phase0-critical, small).
    ua_sb = load_kxm(ua_d, U, U, "uaw", nc.scalar)
    wi_sb = load_kxm(wi_d, EMB, U, "wiw", nc.scalar)

    # ---- persistent intermediates ----
    qb = wp.tile([128, KU, BPC], F32)       # tanh bias (q + ba_w)^T
    xgg = [wp.tile([4, 3 * U], F32, tag=f"xg{g}", name=f"xg{g}") for g in range(2)]
    xgrzg = [wp.tile([4, 2 * U], F32, tag=f"xz{g}", name=f"xz{g}") for g in range(2)]

    def transpose_to(dst, src, nrow, chunks, ident):
        """PE-transpose src [nrow, chunks*128] -> dst [128, chunks, nrow]."""
        pm = pp.tile([128, chunks * nrow], src.dtype, tag="u", name="pm")
        for c in range(chunks):
            nc.tensor.transpose(
                pm[:, c * nrow:(c + 1) * nrow],
                src[0:nrow, c * 128:(c + 1) * 128],
                ident[0:nrow, 0:nrow],
            )
        nc.vector.tensor_copy(dst, pm[:, 0:chunks * nrow])

    def mm8(lhsT, rhs_w, ncol_off, n=U):
        """[BPC, n] = lhsT^T @ rhs_w[:, :, off:off+n], accumulated over KU chunks."""
        ptile = pp.tile([BPC, n], F32, tag="u", name="p8")
        for c in range(KU):
            nc.tensor.matmul(ptile, lhsT[:, c, :],
                             rhs_w[:, c, ncol_off:ncol_off + n],
                             start=(c == 0), stop=(c == KU - 1))
        return ptile

    def mm4(lhsT, g, rhs_w, ncol_off):
        """[4, U] = lhsT[:, :, 4g:4g+4]^T @ rhs_w[:, :, off:off+U]."""
        ptile = pp.tile([4, U], F32, tag="u", name="p4")
        for c in range(KU):
            nc.tensor.matmul(ptile, lhsT[:, c, 4 * g:4 * g + 4],
                             rhs_w[:, c, ncol_off:ncol_off + U],
                             start=(c == 0), stop=(c == KU - 1))
        return ptile

    # ---- phase0-A: q = h @ Ua + ba_u -> qb (critical for first tanh) ----
    hT = wp.tile([128, KU, BPC], F16)
    transpose_to(hT, h016, BPC, KU, id16)
    pq = mm8(hT, ua_sb, 0)
    q16 = wp.tile([BPC, U], F16, tag="q16", name="q16")
    nc.vector.tensor_add(q16, pq, bau8)
    pmq = pp.tile([128, KU * BPC], F16, tag="u", name="pmq")
    for c in range(KU):
        nc.tensor.transpose(pmq[:, c * BPC:(c + 1) * BPC],
                            q16[0:BPC, c * 128:(c + 1) * 128],
                            id16[0:BPC, 0:BPC])
    for c in range(KU):
        nc.vector.tensor_add(qb[:, c, :], pmq[:, c * BPC:(c + 1) * BPC],
                             bawt8[:, c, :])

    # phase0-B part 1: x = inputs @ Wi + bi (xT needed for deferred xg)
    inT = wp.tile([128, 2, BPC], F16)
    transpose_to(inT, inp16, BPC, 2, id16)
    px = pp.tile([BPC, U], F32, tag="u", name="px")
    for c in range(2):
        nc.tensor.matmul(px, inT[:, c, :], wi_sb[:, c, :],
                         start=(c == 0), stop=(c == 1))
    x16 = wp.tile([BPC, U], F16, tag="x16", name="x16")
    nc.vector.tensor_add(x16, px, bi8)
    xT = wp.tile([128, KU, BPC], F16)
    transpose_to(xT, x16, BPC, KU, id16)

    def emit_phase0_rest():
        # xg = x @ kernel + bias ; xgrz = xg_zr + h @ R_zr  (per 4-row group)
        for g in range(2):
            for n in range(3):
                pg = mm4(xT, g, kern_sb, n * U)
                nc.vector.tensor_add(xgg[g][:, n * U:(n + 1) * U], pg,
                                     bg4[:, n * U:(n + 1) * U])
            for n in range(2):
                pr = mm4(hT, g, rec_sb, n * U)
                nc.vector.tensor_add(xgrzg[g][:, n * U:(n + 1) * U], pr,
                                     xgg[g][:, n * U:(n + 1) * U])


    def sigmoid4(dst, pre):
        t1 = gp.tile([4, U], F32, tag="sig_t")
        nc.scalar.activation(t1, pre, AF.Tanh, scale=0.5)
        nc.vector.tensor_scalar(dst, t1, 0.5, 0.5, OP.mult, OP.add)

    def emit_group_post(grp, cvT16, h032, xg):
        """gates, h, out for one 4-row group (partitions 0-3)."""
        xgrz = xgrzg[grp]

        def mm_cv(ncol_off):
            ptile = pp.tile([4, U], F32, tag="u", name="pcv")
            for c in range(KU):
                nc.tensor.matmul(ptile, cvT16[:, c, :],
                                 attk_sb[:, c, ncol_off:ncol_off + U],
                                 start=(c == 0), stop=(c == KU - 1))
            return ptile

        # r gate first: it heads the serial chain (r -> rh -> hbar -> h)
        pcg_r = mm_cv(U)
        rpre = gp.tile([4, U], F32, tag="rpre")
        nc.vector.scalar_tensor_tensor(rpre, pcg_r, 1.0, xgrz[:, U:2 * U],
                                       OP.mult, OP.add)
        # r*h = 0.5*(tanh(rpre/2)+1)*h032; the 0.5 rides the hpre scalar
        tr = gp.tile([4, U], F32, tag="tr")
        nc.scalar.activation(tr, rpre, AF.Tanh, scale=0.5)

        # z gate (only needed at the final blend)
        pcg_z = mm_cv(0)
        zpre = gp.tile([4, U], F32, tag="zpre")
        nc.vector.scalar_tensor_tensor(zpre, pcg_z, 1.0, xgrz[:, 0:U],
                                       OP.mult, OP.add)
        zg = gp.tile([4, U], F32, tag="zg")
        sigmoid4(zg, zpre)

        # rec_h = (r*h) @ Rh
        rh16 = gp.tile([4, U], F16, tag="rh16")
        nc.vector.scalar_tensor_tensor(rh16, tr, 1.0, h032, OP.add, OP.mult)
        rhT = gp.tile([128, KU, 4], F16, tag="rhT")
        transpose_to(rhT, rh16, 4, KU, id16)
        prh = pp.tile([4, U], F32, tag="u", name="prh")
        for c in range(KU):
            nc.tensor.matmul(prh, rhT[:, c, :], rec_sb[:, c, 2 * U:3 * U],
                             start=(c == 0), stop=(c == KU - 1))

        # h_bar  (0.5 folds the (tanh+1) form of the r gate)
        hpre = gp.tile([4, U], F32, tag="hpre")
        nc.vector.scalar_tensor_tensor(hpre, prh, 0.5, xg[:, 2 * U:3 * U],
                                       OP.mult, OP.add)
        pcg_h = mm_cv(2 * U)
        nc.vector.tensor_add(hpre, hpre, pcg_h)
        hbar = gp.tile([4, U], F32, tag="hbar")
        nc.scalar.activation(hbar, hpre, AF.Tanh)

        # h = hbar + z*(h_tm1 - hbar)
        dd = gp.tile([4, U], F32, tag="dd")
        nc.vector.tensor_sub(dd, h032, hbar)
        h_out = gp.tile([4, U], F32, tag="h_out")
        nc.vector.scalar_tensor_tensor(h_out, dd, 1.0, zg, OP.mult, OP.mult)
        nc.vector.tensor_add(h_out, h_out, hbar)
        nc.sync.dma_start(out=h_d[grp * 4:(grp + 1) * 4, :], in_=h_out)

        # out = h @ Wo + bo
        h16 = gp.tile([4, U], F16, tag="h16")
        nc.vector.tensor_copy(h16, h_out)
        hT4 = gp.tile([128, KU, 4], F16, tag="hT4")
        transpose_to(hT4, h16, 4, KU, id16)
        pout = pp.tile([4, U], F32, tag="u", name="pout")
        for c in range(KU):
            nc.tensor.matmul(pout, hT4[:, c, :], wo_sb[:, c, :],
                             start=(c == 0), stop=(c == KU - 1))
        o_out = gp.tile([4, U], F32, tag="o_out")
        nc.vector.tensor_add(o_out, pout, bo4)
        nc.sync.dma_start(out=out_d[grp * 4:(grp + 1) * 4, :], in_=o_out)

    # ---- streaming over batches ----
    # nat[p, j, u] = ctx[b, 16p+j, u]: all t-indexing downstream inherits
    # this scrambled order consistently (softmax is permutation-invariant),
    # so correctness is unaffected.
    cvT16g = None
    pending = []
    phase0_done = False
    for b in range(BPC):
        gi = b % 4
        grp = b // 4
        if gi == 0:
            cvT16g = gp.tile([128, KU, 4], F16, tag="cvT16g")

        natT = nat_pre.pop(b)
        if b + 2 < BPC:
            nat_pre[b + 2] = load_natT(b + 2)

        zp = bp.tile([128, 2], F32, tag="zpb")
        cvPart = bp.tile([128, KU], F32, tag="cvPart")
        cvPartB = bp.tile([128, KU], F32, tag="cvPartB")

        for th in range(2):
            # scores: S^T chunks in PSUM via f8 DoubleRow, tanh -> th8 (f8)
            th8 = thp.tile([128, KU, TH], F8, tag="th8")

            def score_mms(ps_tiles, ms, half):
                for mi, m in enumerate(ms):
                    for c in range(2):
                        nc.tensor.matmul(
                            ps_tiles[mi][:, half * 512:(half + 1) * 512],
                            wa8_sb[:, c, :, m, :],
                            natT[:, 2 * c:2 * c + 2,
                                 th * TH + half * 512:th * TH + (half + 1) * 512],
                            start=(c == 0), stop=(c == 1),
                            perf_mode=DR,
                        )

            ps01 = [pS.tile([128, TH], F32, tag="S", name=f"ps{mm}") for mm in range(2)]
            score_mms(ps01, [0, 1], 0)
            score_mms(ps01, [0, 1], 1)
            for mi, m in enumerate([0, 1]):
                nc.scalar.activation(th8[:, m, :], ps01[mi], AF.Tanh,
                                     scale=1.0 / 16.0, bias=qb[:, m, b:b + 1])
            ps23 = [pS.tile([128, TH], F32, tag="S", name=f"ps{mm + 2}") for mm in range(2)]
            score_mms(ps23, [2, 3], 0)
            score_mms(ps23, [2, 3], 1)
            for mi, m in enumerate([2, 3]):
                nc.scalar.activation(th8[:, m, :], ps23[mi], AF.Tanh,
                                     scale=1.0 / 16.0, bias=qb[:, m, b:b + 1])

            # sc (replicated across partitions) = 16*(S@Va) via DoubleRow f8
            psc = pS.tile([128, TH], F32, tag="S", name="psc")
            for half in range(2):
                for c in range(2):
                    nc.tensor.matmul(psc[:, half * 512:(half + 1) * 512],
                                     va8_sb[:, 2 * c:2 * c + 2, :],
                                     th8[:, 2 * c:2 * c + 2, half * 512:(half + 1) * 512],
                                     start=(c == 0), stop=(c == 1), perf_mode=DR)

            # exp (+accumulate normalizer per partition)
            expRep = erp.tile([128, TH], F16, tag="expRep")
            nc.scalar.activation(expRep, psc, AF.Exp,
                                 scale=1.0 / 16.0, bias=bavr,
                                 accum_out=zp[:, th:th + 1])

            # cv partial on DVE: cv[u] += sum_t natT[u,t]*exp[t]
            dump = erp.tile([128, TH], F16, tag="dump")
            cvdst = cvPart if th == 0 else cvPartB
            for uc in range(KU):
                nc.vector.scalar_tensor_tensor(
                    dump, natT[:, uc, th * TH:(th + 1) * TH], 1.0, expRep,
                    OP.mult, OP.mult, accum_out=cvdst[:, uc:uc + 1])

        # 1/Z and cv^T column for this batch
        zrec = bp.tile([128, 1], F32, tag="zrec")
        nc.vector.tensor_add(zrec, zp[:, 0:1], zp[:, 1:2])
        nc.vector.reciprocal(zrec, zrec)
        cvs = bp.tile([128, KU], F32, tag="cvs")
        nc.vector.tensor_add(cvs, cvPart, cvPartB)
        nc.vector.tensor_scalar(cvT16g[:, :, gi:gi + 1], cvs, zrec, None, OP.mult)

        if gi == 3:
            pending.append((grp, cvT16g))
        if pending and b == 4:
            g0, cv0 = pending.pop(0)
            emit_group_post(g0, cv0, h032g[g0], xgg[g0])
        if not phase0_done and b >= 1:
            emit_phase0_rest()
            phase0_done = True

    while pending:
        g0, cv0 = pending.pop(0)
        emit_group_post(g0, cv0, h032g[g0], xgg[g0])

    es.close()


_PROGRAM = None


def _get_program():
    global _PROGRAM
    if _PROGRAM is None:
        _PROGRAM = _build_program()
    return _PROGRAM


def make_in_maps(inputs, h_tm1, context, Wi, bi, kernel, recurrent_kernel,
                 attention_kernel, bias, Wa, ba_w, Ua, ba_u, Va, ba_v, Wo, bo):
    f32 = lambda x: np.ascontiguousarray(np.asarray(x, dtype=np.float32))
    f16 = lambda x: np.ascontiguousarray(np.asarray(x, dtype=np.float32).astype(np.float16))
    f8np = mybir.dt.np(F8)

    ctxT8 = np.ascontiguousarray(
        np.asarray(context, np.float32).transpose(0, 2, 1).astype(f8np))
    inputs = f32(inputs)
    h_tm1 = f32(h_tm1)

    wa32 = np.asarray(Wa, np.float32) * 16.0
    wa8dr = np.zeros((128, 2, 2, KU, 128), np.float32)
    for c in range(2):
        for i in range(2):
            for mc in range(KU):
                # lhsT[p, i, m] = Wa'[c*256 + i*128 + p, mc*128 + m]
                wa8dr[:, c, i, mc, :] = wa32[c * 256 + i * 128: c * 256 + (i + 1) * 128,
                                             mc * 128:(mc + 1) * 128]
    # va8rep[p, m, j] = 16*Va[m*128+p] for all j (partition-replicated output)
    va16 = (np.asarray(Va, np.float32).reshape(KU, 128) * 16.0)
    va8rep = np.repeat(va16.transpose(1, 0)[:, :, None], 128, axis=2)

    shared = {
        "wa8dr": np.ascontiguousarray(wa8dr.astype(f8np)),
        "va8rep": np.ascontiguousarray(va8rep.astype(f8np)),
        "ident16": np.eye(128, dtype=np.float16),
        "ua16": f16(Ua), "wi16": f16(Wi),
        "kern16": f16(kernel), "rec16": f16(recurrent_kernel),
        "attk16": f16(attention_kernel), "wo16": f16(Wo),
        "bi": f32(bi), "biasg": f32(bias), "ba_u": f32(ba_u),
        "ba_wt8": np.ascontiguousarray(np.repeat(
            np.asarray(ba_w, np.float32).reshape(KU, 128).T[:, :, None], BPC, axis=2)),
        "ba_v1": f32(ba_v).reshape(1, 1),
        "bo": f32(bo),
    }
    in_maps = []
    for i in range(NCORES):
        s = slice(i * BPC, (i + 1) * BPC)
        in_maps.append({
            "ctxT8": ctxT8[s], "inp": inputs[s], "h0": h_tm1[s], **shared,
        })
    return in_maps


def kernel(**inputs):
    from concourse.bass_utils import run_bass_kernel_spmd

    nc = _get_program()
    in_maps = make_in_maps(**inputs)
    res = run_bass_kernel_spmd(nc, in_maps, list(range(NCORES)))
    out = np.concatenate([r["out_o"] for r in res.results], axis=0)
    h = np.concatenate([r["h_o"] for r in res.results], axis=0)
    return out.astype(np.float32), h.astype(np.float32)


if __name__ == "__main__":
    prog = _get_program()
    print("program built OK")


# revision 49
# speedup vs baseline: 1.0088x; 1.0088x over previous
"""Trainium2 Bass kernel for an attention-GRU cell (Bahdanau attention + GRU update).

Computation (per batch row b):
    x   = inputs @ Wi + bi
    xg  = x @ kernel + bias                       (split into x_z, x_r, x_h)
    q   = h_tm1 @ Ua + ba_u
    S   = tanh(context @ Wa + ba_w + q)           [t, U]
    sc  = S @ Va + ba_v                           [t]
    attn = softmax(sc)                            (scores bounded -> no max-sub)
    cv  = sum_t attn * context                    [U]
    cg  = cv @ attention_kernel                   (c_z, c_r, c_h)
    z   = sigmoid(x_z + h@Rz + c_z) ; r = sigmoid(x_r + h@Rr + c_r)
    hb  = tanh(x_h + (r*h)@Rh + c_h)
    h   = z*h_tm1 + (1-z)*hb ; out = h @ Wo + bo

Sharding: batch (64) split across 8 cores, 8 batches/core, weights replicated.
Each core fully independent (no collectives).  Measured ~150us on HW
(v1 baseline: 315-328us).

Design (what got it from 328us to ~150us):
  - context is uploaded HOST-TRANSPOSED as f8e4 [b, U, T] ("ctxT8").  The
    score matmul and the cv reduction both need ctx^T with u on partitions,
    and both already consumed f8 in the 328us version -- so the entire
    on-chip PE-transpose + PSUM->SBUF unpack pipeline (10us PE + 13us
    ACT/DVE per core) vanishes, and HBM context traffic drops 4x
    (33.6MB f32 -> 8.4MB f8 per core, ~2.9us per batch).
  - scores: S^T per m-chunk in PSUM via f8 DoubleRow matmuls (Wa
    pre-scaled x16 on the host, tanh un-scales via scale=1/16);
    tanh outputs f8 th-tiles with per-partition bias qb = (q + ba_w)^T.
  - sc = S@Va via DoubleRow f8 with a PARTITION-REPLICATED Va stationary
    ([128, m, 128] with identical columns): the score row lands replicated
    across all 128 PSUM partitions at no extra PE cost, so exp reads it
    directly and the normalizer accumulates per partition
    (accum_out) -- no partition_broadcast anywhere.
  - cv partials per t-half on the DVE: scalar_tensor_tensor(natT8 x expRep)
    with accum_out per u-chunk.  (f16 natT for DVE 2x and gpsimd STT were
    both tried and are NOT wins: no 2x on HW for this op, and walrus
    rejects TensorScalarPtr on Pool.)
  - steady state is DVE/ACT-bound at ~13us/batch (DVE: 8 STT = 9.8us;
    ACT: 8 tanh + 2 exp = 11.7us); PE ~9us; DMA ~3us.
  - DMA choreography: ctxT8[b0] in two halves first on the gpsimd SWDGE
    queue, then b1, h/inputs, then the big f16 weights (their descriptors
    queue BEHIND the first context tiles on the shared DMA engines instead
    of racing ahead -- this was worth ~15us of head).  ua/wi ride the
    scalar HWDGE queue (needed first for the qb chain), consts on sync.
  - natT pool bufs=3 and expRep/dump bufs=4: with bufs=2 the natT(b+2)
    dma_start blocks the gpsimd queue on the STT(b) anti-dependency and
    stalls the whole pipeline every other batch (~30us total).
  - gates/posts in two 4-row groups (SBUF compute APs cannot start at
    partition 4): group 0's post is emitted after b4 and overlaps the
    stream; group 1's is the tail (~15us serial GRU chain).  The r-gate
    rides tanh algebra: rh = 0.5*(tanh(rpre/2)+1)*h, with the 0.5 folded
    into the hpre scalar slot.  (AF.Sigmoid exists but forces ACT
    activation-table reloads -- slower.)
  - PSUM: score/psc tiles 3x[128,1024]f32 rotating (6 banks) + 2 banks of
    shared small-matmul tiles.

Numerics: rel err ~4.4e-4 vs the f32 reference (budget 2e-2).  f8e4 on
ctx/Wa/Va/tanh-out is smoothed by the 2048-way softmax average; the gate/
output path stays f16/f32.
"""


import sys

if "/opt/trn_rl_repo" not in sys.path:
    sys.path.insert(0, "/opt/trn_rl_repo")

import numpy as np

import concourse.bass as bass
import concourse.mybir as mybir
import concourse.tile as tile
from concourse import bacc

F32 = mybir.dt.float32
F16 = mybir.dt.float16
F8 = mybir.dt.float8e4
AF = mybir.ActivationFunctionType
OP = mybir.AluOpType
DR = mybir.MatmulPerfMode.DoubleRow

B = 64          # total batch
T = 2048        # context length
U = 512         # units
EMB = 256
NCORES = 8
BPC = B // NCORES   # batches per core
KU = U // 128       # 4 k-chunks over units
TC = T // 128       # 16 t-chunks
TH = 1024           # t positions per half


def _build_program():
    nc = bacc.Bacc("TRN2", target_bir_lowering=False, debug=False, num_devices=NCORES)

    # ---- DRAM I/O ----
    ctx_d = nc.dram_tensor("ctxT8", [BPC, U, T], F8, kind="ExternalInput").ap()
    inp_d = nc.dram_tensor("inp", [BPC, EMB], F32, kind="ExternalInput").ap()
    h0_d = nc.dram_tensor("h0", [BPC, U], F32, kind="ExternalInput").ap()

    wa8_d = nc.dram_tensor("wa8dr", [128, 2, 2, KU, 128], F8, kind="ExternalInput").ap()
    va8_d = nc.dram_tensor("va8rep", [128, KU, 128], F8, kind="ExternalInput").ap()
    id16_d = nc.dram_tensor("ident16", [128, 128], F16, kind="ExternalInput").ap()

    ua_d = nc.dram_tensor("ua16", [U, U], F16, kind="ExternalInput").ap()
    wi_d = nc.dram_tensor("wi16", [EMB, U], F16, kind="ExternalInput").ap()
    kern_d = nc.dram_tensor("kern16", [U, 3 * U], F16, kind="ExternalInput").ap()
    rec_d = nc.dram_tensor("rec16", [U, 3 * U], F16, kind="ExternalInput").ap()
    attk_d = nc.dram_tensor("attk16", [U, 3 * U], F16, kind="ExternalInput").ap()
    wo_d = nc.dram_tensor("wo16", [U, U], F16, kind="ExternalInput").ap()

    bi_d = nc.dram_tensor("bi", [U], F32, kind="ExternalInput").ap()
    bg_d = nc.dram_tensor("biasg", [3 * U], F32, kind="ExternalInput").ap()
    bau_d = nc.dram_tensor("ba_u", [U], F32, kind="ExternalInput").ap()
    bawt_d = nc.dram_tensor("ba_wt8", [128, KU, BPC], F32, kind="ExternalInput").ap()
    bav_d = nc.dram_tensor("ba_v1", [1, 1], F32, kind="ExternalInput").ap()
    bo_d = nc.dram_tensor("bo", [U], F32, kind="ExternalInput").ap()

    out_d = nc.dram_tensor("out_o", [BPC, U], F32, kind="ExternalOutput").ap()
    h_d = nc.dram_tensor("h_o", [BPC, U], F32, kind="ExternalOutput").ap()

    with tile.TileContext(nc) as tc:
        _emit(nc, tc, locals())
    nc.compile()
    return nc


def _bcast_rows(ap_1d, rows, cols):
    """DMA source AP replicating a 1-D [cols] dram tensor across `rows` partitions."""
    return bass.AP(ap_1d.tensor, 0, [[0, rows], [1, cols]])


def _emit(nc, tc, d):
    ctx_d, inp_d, h0_d = d["ctx_d"], d["inp_d"], d["h0_d"]
    wa8_d, va8_d, id16_d = d["wa8_d"], d["va8_d"], d["id16_d"]
    ua_d, wi_d, kern_d, rec_d, attk_d, wo_d = (
        d["ua_d"], d["wi_d"], d["kern_d"], d["rec_d"], d["attk_d"], d["wo_d"],
    )
    bi_d, bg_d, bau_d, bawt_d, bav_d, bo_d = (
        d["bi_d"], d["bg_d"], d["bau_d"], d["bawt_d"], d["bav_d"], d["bo_d"],
    )
    out_d, h_d = d["out_d"], d["h_d"]

    from contextlib import ExitStack

    es = ExitStack()
    wp = es.enter_context(tc.tile_pool(name="weights", bufs=1))
    gp = es.enter_context(tc.tile_pool(name="group", bufs=2))
    bp = es.enter_context(tc.tile_pool(name="perbatch", bufs=3))
    ntp = es.enter_context(tc.tile_pool(name="natT", bufs=3))
    thp = es.enter_context(tc.tile_pool(name="th8", bufs=3))
    erp = es.enter_context(tc.tile_pool(name="exprep", bufs=4))
    # PSUM budget 8 banks: pS 2x[128,1024]f32 = 4 banks; pp (shared small:
    # psT f8 [128,2048] and [8,512]f32 tiles) 2 banks; pR psc_rep 2 banks.
    pS = es.enter_context(tc.tile_pool(name="psS", bufs=3, space="PSUM"))
    pp = es.enter_context(tc.tile_pool(name="psT", bufs=2, space="PSUM"))

    # ---- DMA issue order ----

    def load_kxm(dram, rows, cols, tag, engine):
        t = wp.tile([128, rows // 128, cols], F16, tag=tag, name=tag)
        src = bass.AP(dram.tensor, 0, [[cols, 128], [128 * cols, rows // 128], [1, cols]])
        engine.dma_start(out=t, in_=src)
        return t

    def load_natT(bb, engine=None, split=False):
        t = ntp.tile([128, KU, T], F8, tag="natT", name=f"natT{bb}")
        if split:
            # two half-loads so the first scores can start sooner
            for hh in range(2):
                (engine or nc.gpsimd).dma_start(
                    out=t[:, :, hh * TH:(hh + 1) * TH],
                    in_=bass.AP(ctx_d.tensor, bb * U * T + hh * TH,
                                [[T, 128], [128 * T, KU], [1, TH]]))
        else:
            (engine or nc.gpsimd).dma_start(out=t, in_=bass.AP(
                ctx_d.tensor, bb * U * T, [[T, 128], [128 * T, KU], [1, T]]))
        return t

    # batch 0 first on the gpsimd queue, in two halves so the first score
    # matmuls can start as soon as the th0 half lands.
    nat_pre = {0: load_natT(0, split=True)}

    h016 = wp.tile([BPC, U], F16)
    nc.gpsimd.dma_start(out=h016, in_=h0_d)
    inp16 = wp.tile([BPC, EMB], F16)
    nc.gpsimd.dma_start(out=inp16, in_=inp_d)

    nat_pre[1] = load_natT(1)
    # big weights go on the gpsimd SWDGE queue AFTER the first context
    # tiles: their descriptors then queue behind them on the shared DMA
    # engines instead of racing ahead (saves ~15us of head).
    rec_sb = load_kxm(rec_d, U, 3 * U, "recw", nc.gpsimd)
    kern_sb = load_kxm(kern_d, U, 3 * U, "kernw", nc.gpsimd)
    attk_sb = load_kxm(attk_d, U, 3 * U, "attkw", nc.gpsimd)
    wo_sb = load_kxm(wo_d, U, U, "wow", nc.gpsimd)

    # sync queue: small consts first, big rec/attk last.
    id16 = wp.tile([128, 128], F16)
    nc.sync.dma_start(out=id16, in_=id16_d)
    va8_sb = wp.tile([128, KU, 128], F8)
    nc.sync.dma_start(out=va8_sb, in_=va8_d)
    wa8_sb = wp.tile([128, 2, 2, KU, 128], F8)
    nc.sync.dma_start(out=wa8_sb, in_=wa8_d)
    bavr = wp.tile([128, 1], F32)
    nc.sync.dma_start(out=bavr, in_=bass.AP(bav_d.tensor, 0, [[0, 128], [1, 1]]))
    bawt8 = wp.tile([128, KU, BPC], F32)
    nc.sync.dma_start(out=bawt8, in_=bawt_d)
    bau8 = wp.tile([BPC, U], F32)
    nc.sync.dma_start(out=bau8, in_=_bcast_rows(bau_d, BPC, U))
    bi8 = wp.tile([BPC, U], F32)
    nc.sync.dma_start(out=bi8, in_=_bcast_rows(bi_d, BPC, U))
    bg4 = wp.tile([4, 3 * U], F32)
    nc.sync.dma_start(out=bg4, in_=_bcast_rows(bg_d, 4, 3 * U))
    bo4 = wp.tile([4, U], F32)
    nc.sync.dma_start(out=bo4, in_=_bcast_rows(bo_d, 4, U))
    h032g = []
    for g in range(2):
        t = wp.tile([4, U], F32, tag=f"h032g{g}", name=f"h032g{g}")
        nc.sync.dma_start(out=t, in_=h0_d[g * 4:(g + 1) * 4, :])
        h032g.append(t)

    # scalar(ACT) hwdge queue: ua/wi (phase0-critical, small).
    ua_sb = load_kxm(ua_d, U, U, "uaw", nc.scalar)
    wi_sb = load_kxm(wi_d, EMB, U, "wiw", nc.scalar)

    # ---- persistent intermediates ----
    qb = wp.tile([128, KU, BPC], F32)       # tanh bias (q + ba_w)^T
    xgg = [wp.tile([4, 3 * U], F32, tag=f"xg{g}", name=f"xg{g}") for g in range(2)]
    xgrzg = [wp.tile([4, 2 * U], F32, tag=f"xz{g}", name=f"xz{g}") for g in range(2)]

    def transpose_to(dst, src, nrow, chunks, ident):
        """PE-transpose src [nrow, chunks*128] -> dst [128, chunks, nrow]."""
        pm = pp.tile([128, chunks * nrow], src.dtype, tag="u", name="pm")
        for c in range(chunks):
            nc.tensor.transpose(
                pm[:, c * nrow:(c + 1) * nrow],
                src[0:nrow, c * 128:(c + 1) * 128],
                ident[0:nrow, 0:nrow],
            )
        nc.vector.tensor_copy(dst, pm[:, 0:chunks * nrow])

    def mm8(lhsT, rhs_w, ncol_off, n=U):
        """[BPC, n] = lhsT^T @ rhs_w[:, :, off:off+n], accumulated over KU chunks."""
        ptile = pp.tile([BPC, n], F32, tag="u", name="p8")
        for c in range(KU):
            nc.tensor.matmul(ptile, lhsT[:, c, :],
                             rhs_w[:, c, ncol_off:ncol_off + n],
                             start=(c == 0), stop=(c == KU - 1))
        return ptile

    def mm4(lhsT, g, rhs_w, ncol_off):
        """[4, U] = lhsT[:, :, 4g:4g+4]^T @ rhs_w[:, :, off:off+U]."""
        ptile = pp.tile([4, U], F32, tag="u", name="p4")
        for c in range(KU):
            nc.tensor.matmul(ptile, lhsT[:, c, 4 * g:4 * g + 4],
                             rhs_w[:, c, ncol_off:ncol_off + U],
                             start=(c == 0), stop=(c == KU - 1))
        return ptile

    # ---- phase0-A: q = h @ Ua + ba_u -> qb (critical for first tanh) ----
    hT = wp.tile([128, KU, BPC], F16)
    transpose_to(hT, h016, BPC, KU, id16)
    pq = mm8(hT, ua_sb, 0)
    q16 = wp.tile([BPC, U], F16, tag="q16", name="q16")
    nc.vector.tensor_add(q16, pq, bau8)
    pmq = pp.tile([128, KU * BPC], F16, tag="u", name="pmq")
    for c in range(KU):
        nc.tensor.transpose(pmq[:, c * BPC:(c + 1) * BPC],
                            q16[0:BPC, c * 128:(c + 1) * 128],
                            id16[0:BPC, 0:BPC])
    for c in range(KU):
        nc.vector.tensor_add(qb[:, c, :], pmq[:, c * BPC:(c + 1) * BPC],
                             bawt8[:, c, :])

    # phase0-B part 1: x = inputs @ Wi + bi (xT needed for deferred xg)
    inT = wp.tile([128, 2, BPC], F16)
    transpose_to(inT, inp16, BPC, 2, id16)
    px = pp.tile([BPC, U], F32, tag="u", name="px")
    for c in range(2):
        nc.tensor.matmul(px, inT[:, c, :], wi_sb[:, c, :],
                         start=(c == 0), stop=(c == 1))
    x16 = wp.tile([BPC, U], F16, tag="x16", name="x16")
    nc.vector.tensor_add(x16, px, bi8)
    xT = wp.tile([128, KU, BPC], F16)
    transpose_to(xT, x16, BPC, KU, id16)

    def emit_phase0_rest():
        # xg = x @ kernel + bias ; xgrz = xg_zr + h @ R_zr  (per 4-row group)
        for g in range(2):
            for n in range(3):
                pg = mm4(xT, g, kern_sb, n * U)
                nc.vector.tensor_add(xgg[g][:, n * U:(n + 1) * U], pg,
                                     bg4[:, n * U:(n + 1) * U])
            for n in range(2):
                pr = mm4(hT, g, rec_sb, n * U)
                nc.vector.tensor_add(xgrzg[g][:, n * U:(n + 1) * U], pr,
                                     xgg[g][:, n * U:(n + 1) * U])


    def sigmoid4(dst, pre):
        t1 = gp.tile([4, U], F32, tag="sig_t")
        nc.scalar.activation(t1, pre, AF.Tanh, scale=0.5)
        nc.vector.tensor_scalar(dst, t1, 0.5, 0.5, OP.mult, OP.add)

    def emit_group_post(grp, cvT16, h032, xg):
        """gates, h, out for one 4-row group (partitions 0-3)."""
        xgrz = xgrzg[grp]

        def mm_cv(ncol_off):
            ptile = pp.tile([4, U], F32, tag="u", name="pcv")
            for c in range(KU):
                nc.tensor.matmul(ptile, cvT16[:, c, :],
                                 attk_sb[:, c, ncol_off:ncol_off + U],
                                 start=(c == 0), stop=(c == KU - 1))
            return ptile

        # r gate first: it heads the serial chain (r -> rh -> hbar -> h)
        pcg_r = mm_cv(U)
        rpre = gp.tile([4, U], F32, tag="rpre")
        nc.vector.scalar_tensor_tensor(rpre, pcg_r, 1.0, xgrz[:, U:2 * U],
                                       OP.mult, OP.add)
        # r*h = 0.5*(tanh(rpre/2)+1)*h032; the 0.5 rides the hpre scalar
        tr = gp.tile([4, U], F32, tag="tr")
        nc.scalar.activation(tr, rpre, AF.Tanh, scale=0.5)

        # z gate (only needed at the final blend)
        pcg_z = mm_cv(0)
        zpre = gp.tile([4, U], F32, tag="zpre")
        nc.vector.scalar_tensor_tensor(zpre, pcg_z, 1.0, xgrz[:, 0:U],
                                       OP.mult, OP.add)
        zg = gp.tile([4, U], F32, tag="zg")
        sigmoid4(zg, zpre)

        # rec_h = (r*h) @ Rh
        rh16 = gp.tile([4, U], F16, tag="rh16")
        nc.vector.scalar_tensor_tensor(rh16, tr, 1.0, h032, OP.add, OP.mult)
        rhT = gp.tile([128, KU, 4], F16, tag="rhT")
        transpose_to(rhT, rh16, 4, KU, id16)
        prh = pp.tile([4, U], F32, tag="u", name="prh")
        for c in range(KU):
            nc.tensor.matmul(prh, rhT[:, c, :], rec_sb[:, c, 2 * U:3 * U],
                             start=(c == 0), stop=(c == KU - 1))

        # h_bar  (0.5 folds the (tanh+1) form of the r gate)
        hpre = gp.tile([4, U], F32, tag="hpre")
        nc.vector.scalar_tensor_tensor(hpre, prh, 0.5, xg[:, 2 * U:3 * U],
                                       OP.mult, OP.add)
        pcg_h = mm_cv(2 * U)
        nc.vector.tensor_add(hpre, hpre, pcg_h)
        hbar = gp.tile([4, U], F32, tag="hbar")
        nc.scalar.activation(hbar, hpre, AF.Tanh)

        # h = hbar + z*(h_tm1 - hbar)
        dd = gp.tile([4, U], F32, tag="dd")
        nc.vector.tensor_sub(dd, h032, hbar)
        h_out = gp.tile([4, U], F32, tag="h_out")
        nc.vector.scalar_tensor_tensor(h_out, dd, 1.0, zg, OP.mult, OP.mult)
        nc.vector.tensor_add(h_out, h_out, hbar)
        nc.sync.dma_start(out=h_d[grp * 4:(grp + 1) * 4, :], in_=h_out)

        # out = h @ Wo + bo
        h16 = gp.tile([4, U], F16, tag="h16")
        nc.vector.tensor_copy(h16, h_out)
        hT4 = gp.tile([128, KU, 4], F16, tag="hT4")
        transpose_to(hT4, h16, 4, KU, id16)
        pout = pp.tile([4, U], F32, tag="u", name="pout")
        for c in range(KU):
            nc.tensor.matmul(pout, hT4[:, c, :], wo_sb[:, c, :],
                             start=(c == 0), stop=(c == KU - 1))
        o_out = gp.tile([4, U], F32, tag="o_out")
        nc.vector.tensor_add(o_out, pout, bo4)
        nc.sync.dma_start(out=out_d[grp * 4:(grp + 1) * 4, :], in_=o_out)

    # ---- streaming over batches ----
    # nat[p, j, u] = ctx[b, 16p+j, u]: all t-indexing downstream inherits
    # this scrambled order consistently (softmax is permutation-invariant),
    # so correctness is unaffected.
    cvT16g = None
    pending = []
    phase0_done = False
    for b in range(BPC):
        gi = b % 4
        grp = b // 4
        if gi == 0:
            cvT16g = gp.tile([128, KU, 4], F16, tag="cvT16g")

        natT = nat_pre.pop(b)
        if b + 2 < BPC:
            nat_pre[b + 2] = load_natT(b + 2)

        zp = bp.tile([128, 2], F32, tag="zpb")
        cvPart = bp.tile([128, KU], F32, tag="cvPart")
        cvPartB = bp.tile([128, KU], F32, tag="cvPartB")

        for th in range(2):
            # scores: S^T chunks in PSUM via f8 DoubleRow, tanh -> th8 (f8)
            th8 = thp.tile([128, KU, TH], F8, tag="th8")

            def score_mms(ps_tiles, ms, half):
                for mi, m in enumerate(ms):
                    for c in range(2):
                        nc.tensor.matmul(
                            ps_tiles[mi][:, half * 512:(half + 1) * 512],
                            wa8_sb[:, c, :, m, :],
                            natT[:, 2 * c:2 * c + 2,
                                 th * TH + half * 512:th * TH + (half + 1) * 512],
                            start=(c == 0), stop=(c == 1),
                            perf_mode=DR,
                        )

            ps01 = [pS.tile([128, TH], F32, tag="S", name=f"ps{mm}") for mm in range(2)]
            score_mms(ps01, [0, 1], 0)
            score_mms(ps01, [0, 1], 1)
            for mi, m in enumerate([0, 1]):
                nc.scalar.activation(th8[:, m, :], ps01[mi], AF.Tanh,
                                     scale=1.0 / 16.0, bias=qb[:, m, b:b + 1])
            ps23 = [pS.tile([128, TH], F32, tag="S", name=f"ps{mm + 2}") for mm in range(2)]
            score_mms(ps23, [2, 3], 0)
            score_mms(ps23, [2, 3], 1)
            for mi, m in enumerate([2, 3]):
                nc.scalar.activation(th8[:, m, :], ps23[mi], AF.Tanh,
                                     scale=1.0 / 16.0, bias=qb[:, m, b:b + 1])

            # sc (replicated across partitions) = 16*(S@Va) via DoubleRow f8
            psc = pS.tile([128, TH], F32, tag="S", name="psc")
            for half in range(2):
                for c in range(2):
                    nc.tensor.matmul(psc[:, half * 512:(half + 1) * 512],
                                     va8_sb[:, 2 * c:2 * c + 2, :],
                                     th8[:, 2 * c:2 * c + 2, half * 512:(half + 1) * 512],
                                     start=(c == 0), stop=(c == 1), perf_mode=DR)

            # exp (+accumulate normalizer per partition)
            expRep = erp.tile([128, TH], F16, tag="expRep")
            nc.scalar.activation(expRep, psc, AF.Exp,
                                 scale=1.0 / 16.0, bias=bavr,
                                 accum_out=zp[:, th:th + 1])

            # cv partial on DVE: cv[u] += sum_t natT[u,t]*exp[t]
            dump = erp.tile([128, TH], F16, tag="dump")
            cvdst = cvPart if th == 0 else cvPartB
            for uc in range(KU):
                nc.vector.scalar_tensor_tensor(
                    dump, natT[:, uc, th * TH:(th + 1) * TH], 1.0, expRep,
                    OP.mult, OP.mult, accum_out=cvdst[:, uc:uc + 1])

        # 1/Z and cv^T column for this batch
        zrec = bp.tile([128, 1], F32, tag="zrec")
        nc.vector.tensor_add(zrec, zp[:, 0:1], zp[:, 1:2])
        nc.vector.reciprocal(zrec, zrec)
        cvs = bp.tile([128, KU], F32, tag="cvs")
        nc.vector.tensor_add(cvs, cvPart, cvPartB)
        nc.vector.tensor_scalar(cvT16g[:, :, gi:gi + 1], cvs, zrec, None, OP.mult)

        if gi == 3:
            pending.append((grp, cvT16g))
        if pending and b == 4:
            g0, cv0 = pending.pop(0)
            emit_group_post(g0, cv0, h032g[g0], xgg[g0])
        if not phase0_done and b >= 1:
            emit_phase0_rest()
            phase0_done = True

    while pending:
        g0, cv0 = pending.pop(0)
        emit_group_post(g0, cv0, h032g[g0], xgg[g0])

    es.close()


_PROGRAM = None


def _get_program():
    global _PROGRAM
    if _PROGRAM is None:
        _PROGRAM = _build_program()
    return _PROGRAM


def make_in_maps(inputs, h_tm1, context, Wi, bi, kernel, recurrent_kernel,
                 attention_kernel, bias, Wa, ba_w, Ua, ba_u, Va, ba_v, Wo, bo):
    f32 = lambda x: np.ascontiguousarray(np.asarray(x, dtype=np.float32))
    f16 = lambda x: np.ascontiguousarray(np.asarray(x, dtype=np.float32).astype(np.float16))
    f8np = mybir.dt.np(F8)

    ctxT8 = np.ascontiguousarray(
        np.asarray(context, np.float32).transpose(0, 2, 1).astype(f8np))
    inputs = f32(inputs)
    h_tm1 = f32(h_tm1)

    wa32 = np.asarray(Wa, np.float32) * 16.0
    wa8dr = np.zeros((128, 2, 2, KU, 128), np.float32)
    for c in range(2):
        for i in range(2):
            for mc in range(KU):
                # lhsT[p, i, m] = Wa'[c*256 + i*128 + p, mc*128 + m]
                wa8dr[:, c, i, mc, :] = wa32[c * 256 + i * 128: c * 256 + (i + 1) * 128,
                                             mc * 128:(mc + 1) * 128]
    # va8rep[p, m, j] = 16*Va[m*128+p] for all j (partition-replicated output)
    va16 = (np.asarray(Va, np.float32).reshape(KU, 128) * 16.0)
    va8rep = np.repeat(va16.transpose(1, 0)[:, :, None], 128, axis=2)

    shared = {
        "wa8dr": np.ascontiguousarray(wa8dr.astype(f8np)),
        "va8rep": np.ascontiguousarray(va8rep.astype(f8np)),
        "ident16": np.eye(128, dtype=np.float16),
        "ua16": f16(Ua), "wi16": f16(Wi),
        "kern16": f16(kernel), "rec16": f16(recurrent_kernel),
        "attk16": f16(attention_kernel), "wo16": f16(Wo),
        "bi": f32(bi), "biasg": f32(bias), "ba_u": f32(ba_u),
        "ba_wt8": np.ascontiguousarray(np.repeat(
            np.asarray(ba_w, np.float32).reshape(KU, 128).T[:, :, None], BPC, axis=2)),
        "ba_v1": f32(ba_v).reshape(1, 1),
        "bo": f32(bo),
    }
    in_maps = []
    for i in range(NCORES):
        s = slice(i * BPC, (i + 1) * BPC)
        in_maps.append({
            "ctxT8": ctxT8[s], "inp": inputs[s], "h0": h_tm1[s], **shared,
        })
    return in_maps


def kernel(**inputs):
    from concourse.bass_utils import run_bass_kernel_spmd

    nc = _get_program()
    in_maps = make_in_maps(**inputs)
    res = run_bass_kernel_spmd(nc, in_maps, list(range(NCORES)))
    out = np.concatenate([r["out_o"] for r in res.results], axis=0)
    h = np.concatenate([r["h_o"] for r in res.results], axis=0)
    return out.astype(np.float32), h.astype(np.float32)


if __name__ == "__main__":
    prog = _get_program()
    print("program built OK")
